# revision 19
# baseline (speedup 1.0000x reference)
"""DirectedEdgeConv (gnn_message_passing) Trainium2 kernel, 8-core SPMD, bf16.

out[e] = leaky_relu(edge_attr[e] @ Wself^T + b
                    + T_in[src[e]] + T_out[dst[e]], 0.2)
where T_in  = scatter_mean(edge_attr, dst) @ Win^T + b   [node table]
      T_out = scatter_mean(edge_attr, src) @ Wout^T      [node table]

Design (v2, bf16 everywhere):
  Node partition: core c owns nodes [c*NPC, (c+1)*NPC).
  A2 (src-partitioned, src-sorted, block-uniform TB2 tiles/block): one-hot
    PE scatter (inv-count folded into the one-hot) -> own T_out slice ->
    DRAM -> AllGather (bf16) -> full table.
  A1 (dst-partitioned, dst-sorted, TB1): same scatter -> own T_in slice,
    kept in SBUF (bias baked in).  Runs while the AllGather is in flight.
  C (same edge order as A2, transposed stream [d, e]):
    z^T = Wself @ x^T  (PE, no per-tile transposes)
        + T_in[src]    (PE one-hot expansion from SBUF slice)
        + T_out[dst]   (dma_gather transpose=True from full table,
                        4 SWDGE queues round-robin; idx is int16 rebased
                        by -32768 so the full 50k-row table fits)
    LReLU via scalar mul + vector max; y written transposed bf16,
    host restores order/dtype.
"""

import os
import sys

sys.path.insert(0, "/opt/trn_rl_repo")

import numpy as np

import concourse.bacc as bacc
import concourse.bass as bass
import concourse.mybir as mybir
import concourse.tile as tile
from concourse import library_config
from concourse.bass_utils import run_bass_kernel_spmd

P = 128
D = 128
C = 8
HALF = 32768
SUP = 2048           # phase-C supertile (one gather per supertile)
CH = 512             # psum chunk width

F32 = mybir.dt.float32
BF16 = mybir.dt.bfloat16
I16 = mybir.dt.int16


def _cfg_full():
    return dict(E=600000, N=50000)


def _derive(cfg):
    N = cfg["N"]
    NPC = N // C
    NB = (NPC + P - 1) // P
    NBP = NB * P
    return NPC, NB, NBP


def build_kernel(cfg, TB1, TB2, TLO, THI, NSUP_LO, NSUP_HI):
    NPC, NB, NBP = _derive(cfg)
    TROWS = C * NBP
    NT1 = NB * TB1
    NT2 = NB * TB2
    NSUP = NSUP_LO + NSUP_HI
    ESUP = NSUP * SUP
    NTC = ESUP // P

    def tile_block(t):
        # static (core-independent) tile -> src-block map of the C stream
        if t < NSUP_LO * (SUP // P):
            b = t // TLO
        else:
            b = (t - NSUP_LO * (SUP // P)) // THI
        return min(b, NB - 1)  # pad tiles at group end -> clamp (va=-1 anyway)

    nc = bacc.Bacc(None, target_bir_lowering=False, debug=False,
                   num_swdge_queues=4)

    # ---- I/O ----
    xa1 = nc.dram_tensor("xa1", [NB, P, TB1 * D], BF16, kind="ExternalInput")
    va1 = nc.dram_tensor("va1", [P, NT1], F32, kind="ExternalInput")
    iv1 = nc.dram_tensor("iv1", [P, NT1], F32, kind="ExternalInput")
    xa2 = nc.dram_tensor("xa2", [NB, P, TB2 * D], BF16, kind="ExternalInput")
    va2 = nc.dram_tensor("va2", [P, NT2], F32, kind="ExternalInput")
    iv2 = nc.dram_tensor("iv2", [P, NT2], F32, kind="ExternalInput")
    xc = nc.dram_tensor("xc", [P, ESUP], BF16, kind="ExternalInput")
    ohc = nc.dram_tensor("ohc", [P, ESUP], BF16, kind="ExternalInput")
    gidx = nc.dram_tensor("gidx", [P, NSUP * (SUP // 16)], I16, kind="ExternalInput")
    wselfT = nc.dram_tensor("wselfT", [D, D], BF16, kind="ExternalInput")
    winT = nc.dram_tensor("winT", [D, D], BF16, kind="ExternalInput")
    woutT = nc.dram_tensor("woutT", [D, D], BF16, kind="ExternalInput")
    identb = nc.dram_tensor("identb", [D, D], BF16, kind="ExternalInput")
    iota_in = nc.dram_tensor("iota", [P, P], BF16, kind="ExternalInput")
    biascol = nc.dram_tensor("biascol", [P, 1], F32, kind="ExternalInput")
    y = nc.dram_tensor("y", [P, ESUP], BF16, kind="ExternalOutput")
    y2 = nc.dram_tensor("y2", [P, 3 * SUP], BF16, kind="ExternalOutput")

    with tile.TileContext(nc) as tc:
        with (
            tc.tile_pool(name="const", bufs=1) as cpool,
            tc.tile_pool(name="sbuf", bufs=3) as pool,
            tc.tile_pool(name="cstream", bufs=6) as cpool6,
            tc.tile_pool(name="small", bufs=4) as spool,
            tc.tile_pool(name="dram", bufs=1, space="DRAM") as dram,
        ):
            nc.gpsimd.load_library(library_config.mlp)
            iota_t = cpool.tile([P, P], BF16)
            nc.sync.dma_start(out=iota_t[:], in_=iota_in[:])
            wselfT_t = cpool.tile([D, D], BF16)
            nc.sync.dma_start(out=wselfT_t[:], in_=wselfT[:])
            winT_t = cpool.tile([D, D], BF16)
            nc.sync.dma_start(out=winT_t[:], in_=winT[:])
            woutT_t = cpool.tile([D, D], BF16)
            nc.sync.dma_start(out=woutT_t[:], in_=woutT[:])
            identb_t = cpool.tile([D, D], BF16)
            nc.sync.dma_start(out=identb_t[:], in_=identb[:])
            biascol_t = cpool.tile([P, 1], F32)
            nc.sync.dma_start(out=biascol_t[:], in_=biascol[:])
            va1_t = cpool.tile([P, NT1], F32)
            nc.sync.dma_start(out=va1_t[:], in_=va1[:])
            iv1_t = cpool.tile([P, NT1], F32)
            nc.sync.dma_start(out=iv1_t[:], in_=iv1[:])
            va2_t = cpool.tile([P, NT2], F32)
            nc.sync.dma_start(out=va2_t[:], in_=va2[:])
            iv2_t = cpool.tile([P, NT2], F32)
            nc.sync.dma_start(out=iv2_t[:], in_=iv2[:])
            gidx_t = cpool.tile([P, NSUP * (SUP // 16)], I16)
            nc.sync.dma_start(out=gidx_t[:], in_=gidx[:])

            tin_sb = cpool.tile([P, NB * D], BF16)  # T_in slice [n_in_blk, b*D+d]

            cc_in = dram.tile([NBP, D], BF16)
            cc_out = dram.tile([TROWS, D], BF16)

            # ---- phase A (shared): block-uniform one-hot scatter ----
            def phase_a(psum, xa, va_t, iv_t, TB, w_t, out_sb, out_dram, add_bias):
                for b in range(NB):
                    xt = pool.tile([P, TB * D], BF16, tag="xat")
                    nc.sync.dma_start(out=xt[:], in_=xa[b])
                    sp = psum.tile([P, P], F32, tag="SA")
                    for j in range(TB):
                        t = b * TB + j
                        oh = spool.tile([P, P], BF16, tag="oh")
                        nc.vector.tensor_scalar(
                            oh[:], iota_t[:], va_t[:, t : t + 1],
                            iv_t[:, t : t + 1],
                            mybir.AluOpType.is_equal, mybir.AluOpType.mult,
                        )
                        nc.tensor.matmul(
                            sp[:], xt[:, j * D : (j + 1) * D], oh[:],
                            start=(j == 0), stop=(j == TB - 1),
                        )
                    # sp = S^T [d, n]
                    s_sb = spool.tile([P, P], BF16, tag="ssb")
                    nc.scalar.copy(out=s_sb[:], in_=sp[:])
                    tw = psum.tile([P, P], F32, tag="TW")
                    nc.tensor.matmul(tw[:], w_t[:], s_sb[:], start=True, stop=True)
                    # tw = T^T [d', n]
                    t2 = spool.tile([P, P], BF16, tag="t2")
                    if add_bias:
                        nc.vector.tensor_scalar(
                            t2[:], tw[:], biascol_t[:, 0:1], None,
                            mybir.AluOpType.add,
                        )
                    else:
                        nc.scalar.copy(out=t2[:], in_=tw[:])
                    tr = psum.tile([P, P], BF16, tag="TR")
                    nc.tensor.transpose(tr[:], t2[:], identb_t[:])
                    # tr = T [n, d']
                    if out_sb is not None:
                        nc.scalar.copy(out=out_sb[:, b * D : (b + 1) * D], in_=tr[:])
                    else:
                        t3 = spool.tile([P, P], BF16, tag="t3")
                        nc.scalar.copy(out=t3[:], in_=tr[:])
                        nc.sync.dma_start(
                            out=out_dram[b * P : (b + 1) * P, :], in_=t3[:]
                        )

            # A2 first: its AllGather overlaps A1
            with tc.tile_pool(name="psumA", bufs=2, space="PSUM") as psumA:
                phase_a(psumA, xa2, va2_t, iv2_t, TB2, woutT_t, None, cc_in, False)
                nc.gpsimd.collective_compute(
                    "AllGather", mybir.AluOpType.bypass,
                    replica_groups=[list(range(C))],
                    ins=[cc_in.opt()], outs=[cc_out.opt()],
                )
                phase_a(psumA, xa1, va1_t, iv1_t, TB1, winT_t, tin_sb, None, True)

            tc.strict_bb_all_engine_barrier()

            # ---- phase C ----
            psum_cm = tc.tile_pool(name="psumC", bufs=2, space="PSUM")
            psum = psum_cm.__enter__()
            tbl_lo = cc_out[0:HALF, :]
            tbl_hi = cc_out[HALF:TROWS, :]
            SIDX = SUP // 16
            # first 3 supertiles re-emitted at the end: their gathers can
            # race the AllGather landing right after the barrier
            for si, s in enumerate(list(range(NSUP)) + [0, 1, 2]):
                xcs = cpool6.tile([P, SUP], BF16, tag="xcs")
                nc.sync.dma_start(out=xcs[:], in_=xc[:, s * SUP : (s + 1) * SUP])
                ohst = cpool6.tile([P, SUP], BF16, tag="ohst")
                nc.sync.dma_start(out=ohst[:], in_=ohc[:, s * SUP : (s + 1) * SUP])
                go = cpool6.tile([P, SUP], BF16, tag="go")
                nc.gpsimd.dma_gather(
                    out_ap=go[:].rearrange("p (j d) -> p j d", j=SUP // P),
                    in_ap=(tbl_lo if s < NSUP_LO else tbl_hi),
                    idxs_ap=gidx_t[:, s * SIDX : (s + 1) * SIDX],
                    num_idxs=SUP, num_idxs_reg=SUP, elem_size=D,
                    transpose=False, single_packet=False, queue_num=si % 4,
                )
                ysup = pool.tile([P, SUP], BF16, tag="ysup")
                zsb = pool.tile([P, SUP], BF16, tag="zsb")
                for j in range(SUP // P):
                    tg = s * (SUP // P) + j
                    b = tile_block(tg)
                    off = j * P
                    zp = psum.tile([P, P], F32, tag="Z")
                    # z_nat[e, d] = x^T_tile.T @ WselfT  (+ Tin expand + Tout)
                    nc.tensor.matmul(
                        zp[:], xcs[:, off : off + P], wselfT_t[:],
                        start=True, stop=False,
                    )
                    nc.tensor.matmul(
                        zp[:], ohst[:, off : off + P],
                        tin_sb[:, b * D : (b + 1) * D],
                        start=False, stop=False,
                    )
                    nc.tensor.matmul(
                        zp[:], identb_t[:], go[:, off : off + P],
                        start=False, stop=True,
                    )
                    nc.scalar.copy(out=zsb[:, off : off + P], in_=zp[:])
                t1 = spool.tile([P, SUP], BF16, tag="t1")
                nc.scalar.mul(out=t1[:], in_=zsb[:], mul=0.2)
                nc.vector.tensor_max(ysup[:], zsb[:], t1[:])
                if si < NSUP:
                    nc.sync.dma_start(
                        out=y[:, s * SUP : (s + 1) * SUP], in_=ysup[:]
                    )
                else:
                    nc.sync.dma_start(
                        out=y2[:, s * SUP : (s + 1) * SUP], in_=ysup[:]
                    )
            psum_cm.__exit__(None, None, None)

    nc.compile()
    return nc


def prepare_inputs(cfg, edge_attr, edge_index, W_self_w, W_self_b, W_in_w, W_out_w):
    import ml_dtypes
    bf16 = ml_dtypes.bfloat16
    E, N = cfg["E"], cfg["N"]
    NPC, NB, NBP = _derive(cfg)

    edge_attr = np.ascontiguousarray(edge_attr, dtype=np.float32)
    src = np.asarray(edge_index[0], dtype=np.int64)
    dst = np.asarray(edge_index[1], dtype=np.int64)

    wselfT = np.ascontiguousarray(np.asarray(W_self_w, np.float32).T).astype(bf16)
    winT = np.ascontiguousarray(np.asarray(W_in_w, np.float32).T).astype(bf16)
    woutT = np.ascontiguousarray(np.asarray(W_out_w, np.float32).T).astype(bf16)
    identb = np.eye(D, dtype=np.float32).astype(bf16)
    iota = np.tile(np.arange(P, dtype=np.float32)[None, :], (P, 1)).astype(bf16)
    biascol = np.asarray(W_self_b, np.float32).reshape(P, 1)

    ea_bf = edge_attr.astype(bf16)

    # per-node inverse counts (torch_scatter mean semantics: clamp >= 1)
    cnt_dst = np.bincount(dst, minlength=N).astype(np.float32)
    cnt_src = np.bincount(src, minlength=N).astype(np.float32)
    inv_dst = 1.0 / np.maximum(cnt_dst, 1.0)
    inv_src = 1.0 / np.maximum(cnt_src, 1.0)

    # ---- block-uniform A-stream builder ----
    def build_a(node_of_edge, inv_node):
        owner = node_of_edge // NPC
        local = node_of_edge - owner * NPC
        blk = local >> 7
        percore = []
        maxtb = 1
        for c in range(C):
            sel = np.nonzero(owner == c)[0]
            order = sel[np.argsort(local[sel], kind="stable")]
            bcnt = np.bincount(blk[order], minlength=NB)
            maxtb = max(maxtb, int(np.ceil(bcnt.max() / P)))
            percore.append((order, bcnt))
        TB = maxtb
        xs = np.zeros((C, NB, P, TB * D), dtype=bf16)
        va = np.full((C, P, NB * TB), -1.0, dtype=np.float32)
        iv = np.zeros((C, P, NB * TB), dtype=np.float32)
        orders = []
        for c in range(C):
            order, bcnt = percore[c]
            starts = np.zeros(NB, dtype=np.int64)
            np.cumsum(bcnt[:-1], out=starts[1:])
            pos = np.arange(len(order)) - starts[blk[order]]
            slot = blk[order] * (TB * P) + pos  # row within padded stream
            # xs[c, b, p, j*D:(j+1)*D] = edge at (b, j*128+p)
            x_pad = np.zeros((NB * TB * P, D), dtype=bf16)
            x_pad[slot] = ea_bf[order]
            xs[c] = (x_pad.reshape(NB, TB, P, D).transpose(0, 2, 1, 3)
                     .reshape(NB, P, TB * D))
            tilecol = slot // P
            prow = slot % P
            va[c, prow, tilecol] = (local[order] & 127).astype(np.float32)
            iv[c, prow, tilecol] = inv_node[node_of_edge[order]]
            orders.append(order)
        return TB, xs, va, iv, orders

    TB1, xs1, va1, iv1, _ = build_a(dst, inv_dst)
    TB2, xs2, va2, iv2, _ = build_a(src, inv_src)

    trow = lambda n: (n // NPC) * NBP + (n % NPC)
    dst_row = trow(dst)
    hi = (dst_row >= HALF).astype(np.int64)

    # ---- C stream: (half, src-block)-uniform layout ----
    src_owner = src // NPC
    src_local = src - src_owner * NPC
    src_blk = src_local >> 7
    # per (core, block, half) counts -> uniform TLO/THI
    TLO = THI = 1
    perc = []
    for c in range(C):
        sel = np.nonzero(src_owner == c)[0]
        order = sel[np.argsort(src_local[sel] * 2 + hi[sel], kind="stable")]
        blk_o = src_blk[order]
        hi_o = hi[order]
        cl = np.bincount(blk_o[hi_o == 0], minlength=NB)
        ch = np.bincount(blk_o[hi_o == 1], minlength=NB)
        TLO = max(TLO, int(np.ceil(cl.max() / P)))
        THI = max(THI, int(np.ceil(ch.max() / P)))
        perc.append((order, cl, ch))
    NSUP_LO = int(np.ceil(NB * TLO * P / SUP))
    NSUP_HI = int(np.ceil(NB * THI * P / SUP))
    NSUP = NSUP_LO + NSUP_HI
    ESUP = NSUP * SUP
    NTC = ESUP // P
    HIBASE = NSUP_LO * SUP  # stream row where the hi group starts

    xcs = np.zeros((C, P, ESUP), dtype=bf16)
    ohcs = np.zeros((C, P, ESUP), dtype=bf16)
    gidx = np.zeros((C, P, NSUP * (SUP // 16)), dtype=np.int16)
    slots_all = []
    for c in range(C):
        order, cl, ch = perc[c]
        blk_o = src_blk[order]
        hi_o = hi[order]
        slot = np.zeros(len(order), dtype=np.int64)
        for h, cnts, base, TBH in ((0, cl, 0, TLO), (1, ch, HIBASE, THI)):
            idxs = np.nonzero(hi_o == h)[0]
            b_of = blk_o[idxs]
            o2 = np.argsort(b_of, kind="stable")
            bb = b_of[o2]
            st = np.zeros(NB, dtype=np.int64)
            np.cumsum(cnts[:-1], out=st[1:])
            runidx = np.arange(len(idxs)) - st[bb]  # position within block
            ordpos = np.empty(len(idxs), dtype=np.int64)
            ordpos[o2] = runidx
            slot[idxs] = base + b_of * (TBH * P) + ordpos
        x_pad = np.zeros((ESUP, D), dtype=bf16)
        x_pad[slot] = ea_bf[order]
        xcs[c] = x_pad.T
        ohcs[c][(src_local[order] & 127), slot] = 1.0
        gv = np.zeros(ESUP, dtype=np.int64)
        gv[slot] = dst_row[order] - hi[order] * HALF
        gi = np.zeros((16, ESUP // 16), dtype=np.int16)
        gi[np.arange(ESUP) % 16, np.arange(ESUP) // 16] = gv.astype(np.int16)
        gidx[c] = np.tile(gi, (8, 1))
        slots_all.append((order, slot))

    in_maps = []
    for c in range(C):
        in_maps.append(dict(
            xa1=xs1[c], va1=va1[c], iv1=iv1[c],
            xa2=xs2[c], va2=va2[c], iv2=iv2[c],
            xc=xcs[c], ohc=ohcs[c], gidx=gidx[c],
            wselfT=wselfT, winT=winT, woutT=woutT,
            identb=identb, iota=iota, biascol=biascol,
        ))

    def postprocess(results):
        full = np.empty((E, D), dtype=np.float32)
        for c in range(C):
            yT = np.asarray(results[c]["y"], dtype=np.float32)  # [128, ESUP]
            yT[:, : 3 * SUP] = np.asarray(results[c]["y2"], dtype=np.float32)
            # natural per-tile layout: yT[p, t*128+d] = z[edge(t,p), d]
            ynat = (yT.reshape(P, yT.shape[1] // P, P).transpose(1, 0, 2)
                    .reshape(-1, P))
            order, slot = slots_all[c]
            full[order] = ynat[slot]
        return full

    return (TB1, TB2, TLO, THI, NSUP_LO, NSUP_HI), in_maps, postprocess


_NC_CACHE = {}


def run(cfg, inputs, trace=False, trace_kwargs=None):
    params, in_maps, post = prepare_inputs(
        cfg,
        inputs["edge_attr"],
        inputs["edge_index"],
        inputs["W_self_w"],
        inputs["W_self_b"],
        inputs["W_in_w"],
        inputs["W_out_w"],
    )
    key = (tuple(sorted(cfg.items())), params)
    if key not in _NC_CACHE:
        _NC_CACHE[key] = build_kernel(cfg, *params)
    nc = _NC_CACHE[key]
    kw = {}
    if trace:
        kw["trace"] = True
        if trace_kwargs:
            kw.update(trace_kwargs)
    res = run_bass_kernel_spmd(nc, in_maps, core_ids=list(range(C)), **kw)
    return post(res.results), res


def kernel(**inputs) -> np.ndarray:
    out, _ = run(_cfg_full(), inputs)
    return out.astype(np.float32)


# revision 20
# speedup vs baseline: 1.0084x; 1.0084x over previous
"""DirectedEdgeConv (gnn_message_passing) Trainium2 kernel, 8-core SPMD, bf16.

out[e] = leaky_relu(edge_attr[e] @ Wself^T + b
                    + T_in[src[e]] + T_out[dst[e]], 0.2)
where T_in  = scatter_mean(edge_attr, dst) @ Win^T + b   [node table]
      T_out = scatter_mean(edge_attr, src) @ Wout^T      [node table]

Design (v2, bf16 everywhere):
  Node partition: core c owns nodes [c*NPC, (c+1)*NPC).
  A2 (src-partitioned, src-sorted, block-uniform TB2 tiles/block): one-hot
    PE scatter (inv-count folded into the one-hot) -> own T_out slice ->
    DRAM -> AllGather (bf16) -> full table.
  A1 (dst-partitioned, dst-sorted, TB1): same scatter -> own T_in slice,
    kept in SBUF (bias baked in).  Runs while the AllGather is in flight.
  C (same edge order as A2, transposed stream [d, e]):
    z^T = Wself @ x^T  (PE, no per-tile transposes)
        + T_in[src]    (PE one-hot expansion from SBUF slice)
        + T_out[dst]   (dma_gather transpose=True from full table,
                        4 SWDGE queues round-robin; idx is int16 rebased
                        by -32768 so the full 50k-row table fits)
    LReLU via scalar mul + vector max; y written transposed bf16,
    host restores order/dtype.
"""

import os
import sys

sys.path.insert(0, "/opt/trn_rl_repo")

import numpy as np

import concourse.bacc as bacc
import concourse.bass as bass
import concourse.mybir as mybir
import concourse.tile as tile
from concourse import library_config
from concourse.bass_utils import run_bass_kernel_spmd

P = 128
D = 128
C = 8
HALF = 32768
SUP = 2048           # phase-C supertile (one gather per supertile)
CH = 512             # psum chunk width

F32 = mybir.dt.float32
BF16 = mybir.dt.bfloat16
I16 = mybir.dt.int16


def _cfg_full():
    return dict(E=600000, N=50000)


def _derive(cfg):
    N = cfg["N"]
    NPC = N // C
    NB = (NPC + P - 1) // P
    NBP = NB * P
    return NPC, NB, NBP


def build_kernel(cfg, TB1, TB2, TLO, THI, NSUP_LO, NSUP_HI):
    NPC, NB, NBP = _derive(cfg)
    TROWS = C * NBP
    NT1 = NB * TB1
    NT2 = NB * TB2
    NSUP = NSUP_LO + NSUP_HI
    ESUP = NSUP * SUP
    NTC = ESUP // P

    def tile_block(t):
        # static (core-independent) tile -> src-block map of the C stream
        if t < NSUP_LO * (SUP // P):
            b = t // TLO
        else:
            b = (t - NSUP_LO * (SUP // P)) // THI
        return min(b, NB - 1)  # pad tiles at group end -> clamp (va=-1 anyway)

    nc = bacc.Bacc(None, target_bir_lowering=False, debug=False,
                   num_swdge_queues=4)

    # ---- I/O ----
    xa1 = nc.dram_tensor("xa1", [NB, P, TB1 * D], BF16, kind="ExternalInput")
    oha1 = nc.dram_tensor("oha1", [NB, P, TB1 * P], BF16, kind="ExternalInput")
    xa2 = nc.dram_tensor("xa2", [NB, P, TB2 * D], BF16, kind="ExternalInput")
    oha2 = nc.dram_tensor("oha2", [NB, P, TB2 * P], BF16, kind="ExternalInput")
    xc = nc.dram_tensor("xc", [P, ESUP], BF16, kind="ExternalInput")
    ohc = nc.dram_tensor("ohc", [P, ESUP], BF16, kind="ExternalInput")
    gidx = nc.dram_tensor("gidx", [P, NSUP * (SUP // 16)], I16, kind="ExternalInput")
    wselfT = nc.dram_tensor("wselfT", [D, D], BF16, kind="ExternalInput")
    winT = nc.dram_tensor("winT", [D, D], BF16, kind="ExternalInput")
    woutT = nc.dram_tensor("woutT", [D, D], BF16, kind="ExternalInput")
    identb = nc.dram_tensor("identb", [D, D], BF16, kind="ExternalInput")
    iota_in = nc.dram_tensor("iota", [P, P], BF16, kind="ExternalInput")
    biascol = nc.dram_tensor("biascol", [P, 1], F32, kind="ExternalInput")
    y = nc.dram_tensor("y", [P, ESUP], BF16, kind="ExternalOutput")

    with tile.TileContext(nc) as tc:
        with (
            tc.tile_pool(name="const", bufs=1) as cpool,
            tc.tile_pool(name="sbuf", bufs=3) as pool,
            tc.tile_pool(name="cstream", bufs=6) as cpool6,
            tc.tile_pool(name="small", bufs=4) as spool,
            tc.tile_pool(name="dram", bufs=1, space="DRAM") as dram,
        ):
            nc.gpsimd.load_library(library_config.mlp)
            iota_t = cpool.tile([P, P], BF16)
            nc.sync.dma_start(out=iota_t[:], in_=iota_in[:])
            wselfT_t = cpool.tile([D, D], BF16)
            nc.sync.dma_start(out=wselfT_t[:], in_=wselfT[:])
            winT_t = cpool.tile([D, D], BF16)
            nc.sync.dma_start(out=winT_t[:], in_=winT[:])
            woutT_t = cpool.tile([D, D], BF16)
            nc.sync.dma_start(out=woutT_t[:], in_=woutT[:])
            identb_t = cpool.tile([D, D], BF16)
            nc.sync.dma_start(out=identb_t[:], in_=identb[:])
            biascol_t = cpool.tile([P, 1], F32)
            nc.sync.dma_start(out=biascol_t[:], in_=biascol[:])
            gidx_t = cpool.tile([P, NSUP * (SUP // 16)], I16)
            nc.sync.dma_start(out=gidx_t[:], in_=gidx[:])

            tin_sb = cpool.tile([P, NB * D], BF16)  # T_in slice [n_in_blk, b*D+d]

            cc_in = dram.tile([NBP, D], BF16)
            cc_out = dram.tile([TROWS, D], BF16)

            # ---- phase A (shared): block-uniform one-hot scatter ----
            def phase_a(psum, xa, oha, TB, w_t, out_sb, out_dram, add_bias):
                for b in range(NB):
                    xt = pool.tile([P, TB * D], BF16, tag="xat")
                    nc.sync.dma_start(out=xt[:], in_=xa[b])
                    oht = pool.tile([P, TB * P], BF16, tag="oat")
                    nc.sync.dma_start(out=oht[:], in_=oha[b])
                    sp = psum.tile([P, P], F32, tag="SA")
                    for j in range(TB):
                        nc.tensor.matmul(
                            sp[:], xt[:, j * D : (j + 1) * D],
                            oht[:, j * P : (j + 1) * P],
                            start=(j == 0), stop=(j == TB - 1),
                        )
                    # sp = S^T [d, n]
                    s_sb = spool.tile([P, P], BF16, tag="ssb")
                    nc.scalar.copy(out=s_sb[:], in_=sp[:])
                    tw = psum.tile([P, P], F32, tag="TW")
                    nc.tensor.matmul(tw[:], w_t[:], s_sb[:], start=True, stop=True)
                    # tw = T^T [d', n]
                    t2 = spool.tile([P, P], BF16, tag="t2")
                    if add_bias:
                        nc.vector.tensor_scalar(
                            t2[:], tw[:], biascol_t[:, 0:1], None,
                            mybir.AluOpType.add,
                        )
                    else:
                        nc.scalar.copy(out=t2[:], in_=tw[:])
                    tr = psum.tile([P, P], BF16, tag="TR")
                    nc.tensor.transpose(tr[:], t2[:], identb_t[:])
                    # tr = T [n, d']
                    if out_sb is not None:
                        nc.scalar.copy(out=out_sb[:, b * D : (b + 1) * D], in_=tr[:])
                    else:
                        t3 = spool.tile([P, P], BF16, tag="t3")
                        nc.scalar.copy(out=t3[:], in_=tr[:])
                        nc.sync.dma_start(
                            out=out_dram[b * P : (b + 1) * P, :], in_=t3[:]
                        )

            # A2 first: its AllGather overlaps A1
            with tc.tile_pool(name="psumA", bufs=2, space="PSUM") as psumA:
                phase_a(psumA, xa2, oha2, TB2, woutT_t, None, cc_in, False)
                nc.gpsimd.collective_compute(
                    "AllGather", mybir.AluOpType.bypass,
                    replica_groups=[list(range(C))],
                    ins=[cc_in.opt()], outs=[cc_out.opt()],
                )
                phase_a(psumA, xa1, oha1, TB1, winT_t, tin_sb, None, True)

            tc.strict_bb_all_engine_barrier()

            # ---- phase C ----
            psum_cm = tc.tile_pool(name="psumC", bufs=2, space="PSUM")
            psum = psum_cm.__enter__()
            tbl_lo = cc_out[0:HALF, :]
            tbl_hi = cc_out[HALF:TROWS, :]
            SIDX = SUP // 16
            for si, s in enumerate(range(NSUP)):
                xcs = cpool6.tile([P, SUP], BF16, tag="xcs")
                nc.sync.dma_start(out=xcs[:], in_=xc[:, s * SUP : (s + 1) * SUP])
                ohst = cpool6.tile([P, SUP], BF16, tag="ohst")
                nc.sync.dma_start(out=ohst[:], in_=ohc[:, s * SUP : (s + 1) * SUP])
                go = cpool6.tile([P, SUP], BF16, tag="go")
                nc.gpsimd.dma_gather(
                    out_ap=go[:].rearrange("p (j d) -> p j d", j=SUP // P),
                    in_ap=(tbl_lo if s < NSUP_LO else tbl_hi),
                    idxs_ap=gidx_t[:, s * SIDX : (s + 1) * SIDX],
                    num_idxs=SUP, num_idxs_reg=SUP, elem_size=D,
                    transpose=False, single_packet=False, queue_num=si % 4,
                )
                ysup = pool.tile([P, SUP], BF16, tag="ysup")
                zsb = pool.tile([P, SUP], BF16, tag="zsb")
                for j in range(SUP // P):
                    tg = s * (SUP // P) + j
                    b = tile_block(tg)
                    off = j * P
                    zp = psum.tile([P, P], F32, tag="Z")
                    # z_nat[e, d] = x^T_tile.T @ WselfT + Tin expand
                    nc.tensor.matmul(
                        zp[:], xcs[:, off : off + P], wselfT_t[:],
                        start=True, stop=False,
                    )
                    nc.tensor.matmul(
                        zp[:], ohst[:, off : off + P],
                        tin_sb[:, b * D : (b + 1) * D],
                        start=False, stop=True,
                    )
                    nc.scalar.copy(out=zsb[:, off : off + P], in_=zp[:])
                # + T_out (gathered, same [e_p, j*128+d] layout) on DVE
                zadd = pool.tile([P, SUP], BF16, tag="zadd")
                nc.vector.tensor_add(zadd[:], zsb[:], go[:])
                t1 = spool.tile([P, SUP], BF16, tag="t1")
                nc.scalar.mul(out=t1[:], in_=zadd[:], mul=0.2)
                nc.vector.tensor_max(ysup[:], zadd[:], t1[:])
                nc.sync.dma_start(out=y[:, s * SUP : (s + 1) * SUP], in_=ysup[:])
            psum_cm.__exit__(None, None, None)

    nc.compile()
    return nc


def prepare_inputs(cfg, edge_attr, edge_index, W_self_w, W_self_b, W_in_w, W_out_w):
    import ml_dtypes
    bf16 = ml_dtypes.bfloat16
    E, N = cfg["E"], cfg["N"]
    NPC, NB, NBP = _derive(cfg)

    edge_attr = np.ascontiguousarray(edge_attr, dtype=np.float32)
    src = np.asarray(edge_index[0], dtype=np.int64)
    dst = np.asarray(edge_index[1], dtype=np.int64)

    wselfT = np.ascontiguousarray(np.asarray(W_self_w, np.float32).T).astype(bf16)
    winT = np.ascontiguousarray(np.asarray(W_in_w, np.float32).T).astype(bf16)
    woutT = np.ascontiguousarray(np.asarray(W_out_w, np.float32).T).astype(bf16)
    identb = np.eye(D, dtype=np.float32).astype(bf16)
    iota = np.tile(np.arange(P, dtype=np.float32)[None, :], (P, 1)).astype(bf16)
    biascol = np.asarray(W_self_b, np.float32).reshape(P, 1)

    ea_bf = edge_attr.astype(bf16)

    # per-node inverse counts (torch_scatter mean semantics: clamp >= 1)
    cnt_dst = np.bincount(dst, minlength=N).astype(np.float32)
    cnt_src = np.bincount(src, minlength=N).astype(np.float32)
    inv_dst = 1.0 / np.maximum(cnt_dst, 1.0)
    inv_src = 1.0 / np.maximum(cnt_src, 1.0)

    # ---- block-uniform A-stream builder ----
    def build_a(node_of_edge, inv_node):
        owner = node_of_edge // NPC
        local = node_of_edge - owner * NPC
        blk = local >> 7
        percore = []
        maxtb = 1
        for c in range(C):
            sel = np.nonzero(owner == c)[0]
            order = sel[np.argsort(local[sel], kind="stable")]
            bcnt = np.bincount(blk[order], minlength=NB)
            maxtb = max(maxtb, int(np.ceil(bcnt.max() / P)))
            percore.append((order, bcnt))
        TB = maxtb
        xs = np.zeros((C, NB, P, TB * D), dtype=bf16)
        ohs = np.zeros((C, NB, P, TB * P), dtype=bf16)
        orders = []
        for c in range(C):
            order, bcnt = percore[c]
            starts = np.zeros(NB, dtype=np.int64)
            np.cumsum(bcnt[:-1], out=starts[1:])
            pos = np.arange(len(order)) - starts[blk[order]]
            slot = blk[order] * (TB * P) + pos  # row within padded stream
            x_pad = np.zeros((NB * TB * P, D), dtype=bf16)
            x_pad[slot] = ea_bf[order]
            xs[c] = (x_pad.reshape(NB, TB, P, D).transpose(0, 2, 1, 3)
                     .reshape(NB, P, TB * D))
            oh_pad = np.zeros((NB * TB * P, P), dtype=bf16)
            oh_pad[slot, (local[order] & 127)] = inv_node[node_of_edge[order]]
            ohs[c] = (oh_pad.reshape(NB, TB, P, P).transpose(0, 2, 1, 3)
                      .reshape(NB, P, TB * P))
            orders.append(order)
        return TB, xs, ohs, orders

    TB1, xs1, ohs1, _ = build_a(dst, inv_dst)
    TB2, xs2, ohs2, _ = build_a(src, inv_src)

    trow = lambda n: (n // NPC) * NBP + (n % NPC)
    dst_row = trow(dst)
    hi = (dst_row >= HALF).astype(np.int64)

    # ---- C stream: (half, src-block)-uniform layout ----
    src_owner = src // NPC
    src_local = src - src_owner * NPC
    src_blk = src_local >> 7
    # per (core, block, half) counts -> uniform TLO/THI
    TLO = THI = 1
    perc = []
    for c in range(C):
        sel = np.nonzero(src_owner == c)[0]
        order = sel[np.argsort(src_local[sel] * 2 + hi[sel], kind="stable")]
        blk_o = src_blk[order]
        hi_o = hi[order]
        cl = np.bincount(blk_o[hi_o == 0], minlength=NB)
        ch = np.bincount(blk_o[hi_o == 1], minlength=NB)
        TLO = max(TLO, int(np.ceil(cl.max() / P)))
        THI = max(THI, int(np.ceil(ch.max() / P)))
        perc.append((order, cl, ch))
    NSUP_LO = int(np.ceil(NB * TLO * P / SUP))
    NSUP_HI = int(np.ceil(NB * THI * P / SUP))
    NSUP = NSUP_LO + NSUP_HI
    ESUP = NSUP * SUP
    NTC = ESUP // P
    HIBASE = NSUP_LO * SUP  # stream row where the hi group starts

    xcs = np.zeros((C, P, ESUP), dtype=bf16)
    ohcs = np.zeros((C, P, ESUP), dtype=bf16)
    gidx = np.zeros((C, P, NSUP * (SUP // 16)), dtype=np.int16)
    slots_all = []
    for c in range(C):
        order, cl, ch = perc[c]
        blk_o = src_blk[order]
        hi_o = hi[order]
        slot = np.zeros(len(order), dtype=np.int64)
        for h, cnts, base, TBH in ((0, cl, 0, TLO), (1, ch, HIBASE, THI)):
            idxs = np.nonzero(hi_o == h)[0]
            b_of = blk_o[idxs]
            o2 = np.argsort(b_of, kind="stable")
            bb = b_of[o2]
            st = np.zeros(NB, dtype=np.int64)
            np.cumsum(cnts[:-1], out=st[1:])
            runidx = np.arange(len(idxs)) - st[bb]  # position within block
            ordpos = np.empty(len(idxs), dtype=np.int64)
            ordpos[o2] = runidx
            slot[idxs] = base + b_of * (TBH * P) + ordpos
        x_pad = np.zeros((ESUP, D), dtype=bf16)
        x_pad[slot] = ea_bf[order]
        xcs[c] = x_pad.T
        ohcs[c][(src_local[order] & 127), slot] = 1.0
        gv = np.zeros(ESUP, dtype=np.int64)
        gv[slot] = dst_row[order] - hi[order] * HALF
        gi = np.zeros((16, ESUP // 16), dtype=np.int16)
        gi[np.arange(ESUP) % 16, np.arange(ESUP) // 16] = gv.astype(np.int16)
        gidx[c] = np.tile(gi, (8, 1))
        slots_all.append((order, slot))

    in_maps = []
    for c in range(C):
        in_maps.append(dict(
            xa1=xs1[c], oha1=ohs1[c],
            xa2=xs2[c], oha2=ohs2[c],
            xc=xcs[c], ohc=ohcs[c], gidx=gidx[c],
            wselfT=wselfT, winT=winT, woutT=woutT,
            identb=identb, iota=iota, biascol=biascol,
        ))

    def postprocess(results):
        full = np.empty((E, D), dtype=np.float32)
        for c in range(C):
            yT = np.asarray(results[c]["y"], dtype=np.float32)  # [128, ESUP]
            # natural per-tile layout: yT[p, t*128+d] = z[edge(t,p), d]
            ynat = (yT.reshape(P, yT.shape[1] // P, P).transpose(1, 0, 2)
                    .reshape(-1, P))
            order, slot = slots_all[c]
            full[order] = ynat[slot]
        return full

    return (TB1, TB2, TLO, THI, NSUP_LO, NSUP_HI), in_maps, postprocess


_NC_CACHE = {}


def run(cfg, inputs, trace=False, trace_kwargs=None):
    params, in_maps, post = prepare_inputs(
        cfg,
        inputs["edge_attr"],
        inputs["edge_index"],
        inputs["W_self_w"],
        inputs["W_self_b"],
        inputs["W_in_w"],
        inputs["W_out_w"],
    )
    key = (tuple(sorted(cfg.items())), params)
    if key not in _NC_CACHE:
        _NC_CACHE[key] = build_kernel(cfg, *params)
    nc = _NC_CACHE[key]
    kw = {}
    if trace:
        kw["trace"] = True
        if trace_kwargs:
            kw.update(trace_kwargs)
    res = run_bass_kernel_spmd(nc, in_maps, core_ids=list(range(C)), **kw)
    return post(res.results), res


def kernel(**inputs) -> np.ndarray:
    out, _ = run(_cfg_full(), inputs)
    return out.astype(np.float32)


# revision 21
# speedup vs baseline: 1.0149x; 1.0064x over previous
"""DirectedEdgeConv (gnn_message_passing) Trainium2 kernel, 8-core SPMD, bf16.

out[e] = leaky_relu(edge_attr[e] @ Wself^T + b
                    + T_in[src[e]] + T_out[dst[e]], 0.2)
where T_in  = scatter_mean(edge_attr, dst) @ Win^T + b   [node table]
      T_out = scatter_mean(edge_attr, src) @ Wout^T      [node table]

Design (v2, bf16 everywhere):
  Node partition: core c owns nodes [c*NPC, (c+1)*NPC).
  A2 (src-partitioned, src-sorted, block-uniform TB2 tiles/block): one-hot
    PE scatter (inv-count folded into the one-hot) -> own T_out slice ->
    DRAM -> AllGather (bf16) -> full table.
  A1 (dst-partitioned, dst-sorted, TB1): same scatter -> own T_in slice,
    kept in SBUF (bias baked in).  Runs while the AllGather is in flight.
  C (same edge order as A2, transposed stream [d, e]):
    z^T = Wself @ x^T  (PE, no per-tile transposes)
        + T_in[src]    (PE one-hot expansion from SBUF slice)
        + T_out[dst]   (dma_gather transpose=True from full table,
                        4 SWDGE queues round-robin; idx is int16 rebased
                        by -32768 so the full 50k-row table fits)
    LReLU via scalar mul + vector max; y written transposed bf16,
    host restores order/dtype.
"""

import os
import sys

sys.path.insert(0, "/opt/trn_rl_repo")

import numpy as np

import concourse.bacc as bacc
import concourse.bass as bass
import concourse.mybir as mybir
import concourse.tile as tile
from concourse import library_config
from concourse.bass_utils import run_bass_kernel_spmd

P = 128
D = 128
C = 8
HALF = 32768
SUP = 2048           # phase-C supertile (one gather per supertile)
CH = 512             # psum chunk width

F32 = mybir.dt.float32
BF16 = mybir.dt.bfloat16
I16 = mybir.dt.int16


def _cfg_full():
    return dict(E=600000, N=50000)


def _derive(cfg):
    N = cfg["N"]
    NPC = N // C
    NB = (NPC + P - 1) // P
    NBP = NB * P
    return NPC, NB, NBP


def build_kernel(cfg, TB1, TB2, TLO, THI, NSUP_LO, NSUP_HI):
    NPC, NB, NBP = _derive(cfg)
    TROWS = C * NBP
    NT1 = NB * TB1
    NT2 = NB * TB2
    NSUP = NSUP_LO + NSUP_HI
    ESUP = NSUP * SUP
    NTC = ESUP // P

    def tile_block(t):
        # static (core-independent) tile -> src-block map of the C stream
        if t < NSUP_LO * (SUP // P):
            b = t // TLO
        else:
            b = (t - NSUP_LO * (SUP // P)) // THI
        return min(b, NB - 1)  # pad tiles at group end -> clamp (va=-1 anyway)

    nc = bacc.Bacc(None, target_bir_lowering=False, debug=False,
                   num_swdge_queues=4)

    # ---- I/O ----
    xa1 = nc.dram_tensor("xa1", [NB, P, TB1 * D], BF16, kind="ExternalInput")
    oha1 = nc.dram_tensor("oha1", [NB, P, TB1 * P], BF16, kind="ExternalInput")
    xa2 = nc.dram_tensor("xa2", [NB, P, TB2 * D], BF16, kind="ExternalInput")
    oha2 = nc.dram_tensor("oha2", [NB, P, TB2 * P], BF16, kind="ExternalInput")
    xc = nc.dram_tensor("xc", [P, ESUP], BF16, kind="ExternalInput")
    ohc = nc.dram_tensor("ohc", [P, ESUP], BF16, kind="ExternalInput")
    gidx = nc.dram_tensor("gidx", [P, NSUP * (SUP // 16)], I16, kind="ExternalInput")
    wselfT = nc.dram_tensor("wselfT", [D, D], BF16, kind="ExternalInput")
    winT = nc.dram_tensor("winT", [D, D], BF16, kind="ExternalInput")
    woutT = nc.dram_tensor("woutT", [D, D], BF16, kind="ExternalInput")
    identb = nc.dram_tensor("identb", [D, D], BF16, kind="ExternalInput")
    iota_in = nc.dram_tensor("iota", [P, P], BF16, kind="ExternalInput")
    biascol = nc.dram_tensor("biascol", [P, 1], F32, kind="ExternalInput")
    y = nc.dram_tensor("y", [P, ESUP], BF16, kind="ExternalOutput")

    with tile.TileContext(nc) as tc:
        with (
            tc.tile_pool(name="const", bufs=1) as cpool,
            tc.tile_pool(name="sbuf", bufs=3) as pool,
            tc.tile_pool(name="cstream", bufs=6) as cpool6,
            tc.tile_pool(name="small", bufs=4) as spool,
            tc.tile_pool(name="dram", bufs=1, space="DRAM") as dram,
        ):
            nc.gpsimd.load_library(library_config.mlp)
            iota_t = cpool.tile([P, P], BF16)
            nc.sync.dma_start(out=iota_t[:], in_=iota_in[:])
            wselfT_t = cpool.tile([D, D], BF16)
            nc.sync.dma_start(out=wselfT_t[:], in_=wselfT[:])
            winT_t = cpool.tile([D, D], BF16)
            nc.sync.dma_start(out=winT_t[:], in_=winT[:])
            woutT_t = cpool.tile([D, D], BF16)
            nc.sync.dma_start(out=woutT_t[:], in_=woutT[:])
            identb_t = cpool.tile([D, D], BF16)
            nc.sync.dma_start(out=identb_t[:], in_=identb[:])
            biascol_t = cpool.tile([P, 1], F32)
            nc.sync.dma_start(out=biascol_t[:], in_=biascol[:])
            gidx_t = cpool.tile([P, NSUP * (SUP // 16)], I16)
            nc.sync.dma_start(out=gidx_t[:], in_=gidx[:])

            tin_sb = cpool.tile([P, NB * D], BF16)  # T_in slice [n_in_blk, b*D+d]

            cc_in = dram.tile([NBP, D], BF16)
            cc_out = dram.tile([TROWS, D], BF16)

            # ---- phase A (shared): block-uniform one-hot scatter ----
            def phase_a(psum, xa, oha, TB, w_t, out_sb, out_dram, add_bias):
                for b in range(NB):
                    xt = pool.tile([P, TB * D], BF16, tag="xat")
                    nc.sync.dma_start(out=xt[:], in_=xa[b])
                    oht = pool.tile([P, TB * P], BF16, tag="oat")
                    nc.sync.dma_start(out=oht[:], in_=oha[b])
                    sp = psum.tile([P, P], F32, tag="SA")
                    for j in range(TB):
                        nc.tensor.matmul(
                            sp[:], xt[:, j * D : (j + 1) * D],
                            oht[:, j * P : (j + 1) * P],
                            start=(j == 0), stop=(j == TB - 1),
                        )
                    # sp = S^T [d, n]
                    s_sb = spool.tile([P, P], BF16, tag="ssb")
                    nc.scalar.copy(out=s_sb[:], in_=sp[:])
                    tw = psum.tile([P, P], F32, tag="TW")
                    nc.tensor.matmul(tw[:], w_t[:], s_sb[:], start=True, stop=True)
                    # tw = T^T [d', n]
                    t2 = spool.tile([P, P], BF16, tag="t2")
                    if add_bias:
                        nc.vector.tensor_scalar(
                            t2[:], tw[:], biascol_t[:, 0:1], None,
                            mybir.AluOpType.add,
                        )
                    else:
                        nc.scalar.copy(out=t2[:], in_=tw[:])
                    tr = psum.tile([P, P], BF16, tag="TR")
                    nc.tensor.transpose(tr[:], t2[:], identb_t[:])
                    # tr = T [n, d']
                    if out_sb is not None:
                        nc.scalar.copy(out=out_sb[:, b * D : (b + 1) * D], in_=tr[:])
                    else:
                        t3 = spool.tile([P, P], BF16, tag="t3")
                        nc.scalar.copy(out=t3[:], in_=tr[:])
                        nc.sync.dma_start(
                            out=out_dram[b * P : (b + 1) * P, :], in_=t3[:]
                        )

            # A2 first: its AllGather overlaps A1
            with tc.tile_pool(name="psumA", bufs=2, space="PSUM") as psumA:
                phase_a(psumA, xa2, oha2, TB2, woutT_t, None, cc_in, False)
                nc.gpsimd.collective_compute(
                    "AllGather", mybir.AluOpType.bypass,
                    replica_groups=[list(range(C))],
                    ins=[cc_in.opt()], outs=[cc_out.opt()],
                )
                phase_a(psumA, xa1, oha1, TB1, winT_t, tin_sb, None, True)

            # ---- phase C ----
            psum_cm = tc.tile_pool(name="psumC", bufs=2, space="PSUM")
            psum = psum_cm.__enter__()
            tbl_lo = cc_out[0:HALF, :]
            tbl_hi = cc_out[HALF:TROWS, :]
            SIDX = SUP // 16
            for si, s in enumerate(range(NSUP)):
                xcs = cpool6.tile([P, SUP], BF16, tag="xcs")
                nc.sync.dma_start(out=xcs[:], in_=xc[:, s * SUP : (s + 1) * SUP])
                ohst = cpool6.tile([P, SUP], BF16, tag="ohst")
                nc.sync.dma_start(out=ohst[:], in_=ohc[:, s * SUP : (s + 1) * SUP])
                go = cpool6.tile([P, SUP], BF16, tag="go")
                nc.gpsimd.dma_gather(
                    out_ap=go[:].rearrange("p (j d) -> p j d", j=SUP // P),
                    in_ap=(tbl_lo if s < NSUP_LO else tbl_hi),
                    idxs_ap=gidx_t[:, s * SIDX : (s + 1) * SIDX],
                    num_idxs=SUP, num_idxs_reg=SUP, elem_size=D,
                    transpose=False, single_packet=False, queue_num=si % 4,
                )
                ysup = pool.tile([P, SUP], BF16, tag="ysup")
                zsb = pool.tile([P, SUP], BF16, tag="zsb")
                for j in range(SUP // P):
                    tg = s * (SUP // P) + j
                    b = tile_block(tg)
                    off = j * P
                    zp = psum.tile([P, P], F32, tag="Z")
                    # z_nat[e, d] = x^T_tile.T @ WselfT + Tin expand
                    nc.tensor.matmul(
                        zp[:], xcs[:, off : off + P], wselfT_t[:],
                        start=True, stop=False,
                    )
                    nc.tensor.matmul(
                        zp[:], ohst[:, off : off + P],
                        tin_sb[:, b * D : (b + 1) * D],
                        start=False, stop=True,
                    )
                    nc.scalar.copy(out=zsb[:, off : off + P], in_=zp[:])
                # + T_out (gathered, same [e_p, j*128+d] layout) on DVE
                zadd = pool.tile([P, SUP], BF16, tag="zadd")
                nc.vector.tensor_add(zadd[:], zsb[:], go[:])
                t1 = spool.tile([P, SUP], BF16, tag="t1")
                nc.scalar.mul(out=t1[:], in_=zadd[:], mul=0.2)
                nc.vector.tensor_max(ysup[:], zadd[:], t1[:])
                nc.sync.dma_start(out=y[:, s * SUP : (s + 1) * SUP], in_=ysup[:])
            psum_cm.__exit__(None, None, None)

    nc.compile()
    return nc


def prepare_inputs(cfg, edge_attr, edge_index, W_self_w, W_self_b, W_in_w, W_out_w):
    import ml_dtypes
    bf16 = ml_dtypes.bfloat16
    E, N = cfg["E"], cfg["N"]
    NPC, NB, NBP = _derive(cfg)

    edge_attr = np.ascontiguousarray(edge_attr, dtype=np.float32)
    src = np.asarray(edge_index[0], dtype=np.int64)
    dst = np.asarray(edge_index[1], dtype=np.int64)

    wselfT = np.ascontiguousarray(np.asarray(W_self_w, np.float32).T).astype(bf16)
    winT = np.ascontiguousarray(np.asarray(W_in_w, np.float32).T).astype(bf16)
    woutT = np.ascontiguousarray(np.asarray(W_out_w, np.float32).T).astype(bf16)
    identb = np.eye(D, dtype=np.float32).astype(bf16)
    iota = np.tile(np.arange(P, dtype=np.float32)[None, :], (P, 1)).astype(bf16)
    biascol = np.asarray(W_self_b, np.float32).reshape(P, 1)

    ea_bf = edge_attr.astype(bf16)

    # per-node inverse counts (torch_scatter mean semantics: clamp >= 1)
    cnt_dst = np.bincount(dst, minlength=N).astype(np.float32)
    cnt_src = np.bincount(src, minlength=N).astype(np.float32)
    inv_dst = 1.0 / np.maximum(cnt_dst, 1.0)
    inv_src = 1.0 / np.maximum(cnt_src, 1.0)

    # ---- block-uniform A-stream builder ----
    def build_a(node_of_edge, inv_node):
        owner = node_of_edge // NPC
        local = node_of_edge - owner * NPC
        blk = local >> 7
        percore = []
        maxtb = 1
        for c in range(C):
            sel = np.nonzero(owner == c)[0]
            order = sel[np.argsort(local[sel], kind="stable")]
            bcnt = np.bincount(blk[order], minlength=NB)
            maxtb = max(maxtb, int(np.ceil(bcnt.max() / P)))
            percore.append((order, bcnt))
        TB = maxtb
        xs = np.zeros((C, NB, P, TB * D), dtype=bf16)
        ohs = np.zeros((C, NB, P, TB * P), dtype=bf16)
        orders = []
        for c in range(C):
            order, bcnt = percore[c]
            starts = np.zeros(NB, dtype=np.int64)
            np.cumsum(bcnt[:-1], out=starts[1:])
            pos = np.arange(len(order)) - starts[blk[order]]
            slot = blk[order] * (TB * P) + pos  # row within padded stream
            x_pad = np.zeros((NB * TB * P, D), dtype=bf16)
            x_pad[slot] = ea_bf[order]
            xs[c] = (x_pad.reshape(NB, TB, P, D).transpose(0, 2, 1, 3)
                     .reshape(NB, P, TB * D))
            oh_pad = np.zeros((NB * TB * P, P), dtype=bf16)
            oh_pad[slot, (local[order] & 127)] = inv_node[node_of_edge[order]]
            ohs[c] = (oh_pad.reshape(NB, TB, P, P).transpose(0, 2, 1, 3)
                      .reshape(NB, P, TB * P))
            orders.append(order)
        return TB, xs, ohs, orders

    TB1, xs1, ohs1, _ = build_a(dst, inv_dst)
    TB2, xs2, ohs2, _ = build_a(src, inv_src)

    trow = lambda n: (n // NPC) * NBP + (n % NPC)
    dst_row = trow(dst)
    hi = (dst_row >= HALF).astype(np.int64)

    # ---- C stream: (half, src-block)-uniform layout ----
    src_owner = src // NPC
    src_local = src - src_owner * NPC
    src_blk = src_local >> 7
    # per (core, block, half) counts -> uniform TLO/THI
    TLO = THI = 1
    perc = []
    for c in range(C):
        sel = np.nonzero(src_owner == c)[0]
        order = sel[np.argsort(src_local[sel] * 2 + hi[sel], kind="stable")]
        blk_o = src_blk[order]
        hi_o = hi[order]
        cl = np.bincount(blk_o[hi_o == 0], minlength=NB)
        ch = np.bincount(blk_o[hi_o == 1], minlength=NB)
        TLO = max(TLO, int(np.ceil(cl.max() / P)))
        THI = max(THI, int(np.ceil(ch.max() / P)))
        perc.append((order, cl, ch))
    NSUP_LO = int(np.ceil(NB * TLO * P / SUP))
    NSUP_HI = int(np.ceil(NB * THI * P / SUP))
    NSUP = NSUP_LO + NSUP_HI
    ESUP = NSUP * SUP
    NTC = ESUP // P
    HIBASE = NSUP_LO * SUP  # stream row where the hi group starts

    xcs = np.zeros((C, P, ESUP), dtype=bf16)
    ohcs = np.zeros((C, P, ESUP), dtype=bf16)
    gidx = np.zeros((C, P, NSUP * (SUP // 16)), dtype=np.int16)
    slots_all = []
    for c in range(C):
        order, cl, ch = perc[c]
        blk_o = src_blk[order]
        hi_o = hi[order]
        slot = np.zeros(len(order), dtype=np.int64)
        for h, cnts, base, TBH in ((0, cl, 0, TLO), (1, ch, HIBASE, THI)):
            idxs = np.nonzero(hi_o == h)[0]
            b_of = blk_o[idxs]
            o2 = np.argsort(b_of, kind="stable")
            bb = b_of[o2]
            st = np.zeros(NB, dtype=np.int64)
            np.cumsum(cnts[:-1], out=st[1:])
            runidx = np.arange(len(idxs)) - st[bb]  # position within block
            ordpos = np.empty(len(idxs), dtype=np.int64)
            ordpos[o2] = runidx
            slot[idxs] = base + b_of * (TBH * P) + ordpos
        x_pad = np.zeros((ESUP, D), dtype=bf16)
        x_pad[slot] = ea_bf[order]
        xcs[c] = x_pad.T
        ohcs[c][(src_local[order] & 127), slot] = 1.0
        gv = np.zeros(ESUP, dtype=np.int64)
        gv[slot] = dst_row[order] - hi[order] * HALF
        gi = np.zeros((16, ESUP // 16), dtype=np.int16)
        gi[np.arange(ESUP) % 16, np.arange(ESUP) // 16] = gv.astype(np.int16)
        gidx[c] = np.tile(gi, (8, 1))
        slots_all.append((order, slot))

    in_maps = []
    for c in range(C):
        in_maps.append(dict(
            xa1=xs1[c], oha1=ohs1[c],
            xa2=xs2[c], oha2=ohs2[c],
            xc=xcs[c], ohc=ohcs[c], gidx=gidx[c],
            wselfT=wselfT, winT=winT, woutT=woutT,
            identb=identb, iota=iota, biascol=biascol,
        ))

    def postprocess(results):
        full = np.empty((E, D), dtype=np.float32)
        for c in range(C):
            yT = np.asarray(results[c]["y"], dtype=np.float32)  # [128, ESUP]
            # natural per-tile layout: yT[p, t*128+d] = z[edge(t,p), d]
            ynat = (yT.reshape(P, yT.shape[1] // P, P).transpose(1, 0, 2)
                    .reshape(-1, P))
            order, slot = slots_all[c]
            full[order] = ynat[slot]
        return full

    return (TB1, TB2, TLO, THI, NSUP_LO, NSUP_HI), in_maps, postprocess


_NC_CACHE = {}


def run(cfg, inputs, trace=False, trace_kwargs=None):
    params, in_maps, post = prepare_inputs(
        cfg,
        inputs["edge_attr"],
        inputs["edge_index"],
        inputs["W_self_w"],
        inputs["W_self_b"],
        inputs["W_in_w"],
        inputs["W_out_w"],
    )
    key = (tuple(sorted(cfg.items())), params)
    if key not in _NC_CACHE:
        _NC_CACHE[key] = build_kernel(cfg, *params)
    nc = _NC_CACHE[key]
    kw = {}
    if trace:
        kw["trace"] = True
        if trace_kwargs:
            kw.update(trace_kwargs)
    res = run_bass_kernel_spmd(nc, in_maps, core_ids=list(range(C)), **kw)
    return post(res.results), res


def kernel(**inputs) -> np.ndarray:
    out, _ = run(_cfg_full(), inputs)
    return out.astype(np.float32)


# revision 22
# speedup vs baseline: 1.0246x; 1.0096x over previous
"""DirectedEdgeConv (gnn_message_passing) Trainium2 kernel, 8-core SPMD, bf16.

out[e] = leaky_relu(edge_attr[e] @ Wself^T + b
                    + T_in[src[e]] + T_out[dst[e]], 0.2)
where T_in  = scatter_mean(edge_attr, dst) @ Win^T + b   [node table]
      T_out = scatter_mean(edge_attr, src) @ Wout^T      [node table]

Design (v2, bf16 everywhere):
  Node partition: core c owns nodes [c*NPC, (c+1)*NPC).
  A2 (src-partitioned, src-sorted, block-uniform TB2 tiles/block): one-hot
    PE scatter (inv-count folded into the one-hot) -> own T_out slice ->
    DRAM -> AllGather (bf16) -> full table.
  A1 (dst-partitioned, dst-sorted, TB1): same scatter -> own T_in slice,
    kept in SBUF (bias baked in).  Runs while the AllGather is in flight.
  C (same edge order as A2, transposed stream [d, e]):
    z^T = Wself @ x^T  (PE, no per-tile transposes)
        + T_in[src]    (PE one-hot expansion from SBUF slice)
        + T_out[dst]   (dma_gather transpose=True from full table,
                        4 SWDGE queues round-robin; idx is int16 rebased
                        by -32768 so the full 50k-row table fits)
    LReLU via scalar mul + vector max; y written transposed bf16,
    host restores order/dtype.
"""

import os
import sys

sys.path.insert(0, "/opt/trn_rl_repo")

import numpy as np

import concourse.bacc as bacc
import concourse.bass as bass
import concourse.mybir as mybir
import concourse.tile as tile
from concourse import library_config
from concourse.bass_utils import run_bass_kernel_spmd

P = 128
D = 128
C = 8
HALF = 32768
SUP = 2048           # phase-C supertile (one gather per supertile)
CH = 512             # psum chunk width

F32 = mybir.dt.float32
BF16 = mybir.dt.bfloat16
I16 = mybir.dt.int16


def _cfg_full():
    return dict(E=600000, N=50000)


def _derive(cfg):
    N = cfg["N"]
    NPC = N // C
    NB = (NPC + P - 1) // P
    NBP = NB * P
    return NPC, NB, NBP


def build_kernel(cfg, TB1, TB2, TLO, THI, NSUP_LO, NSUP_HI):
    NPC, NB, NBP = _derive(cfg)
    TROWS = C * NBP
    NT1 = NB * TB1
    NT2 = NB * TB2
    NSUP = NSUP_LO + NSUP_HI
    ESUP = NSUP * SUP
    NTC = ESUP // P

    def tile_block(t):
        # static (core-independent) tile -> src-block map of the C stream
        if t < NSUP_LO * (SUP // P):
            b = t // TLO
        else:
            b = (t - NSUP_LO * (SUP // P)) // THI
        return min(b, NB - 1)  # pad tiles at group end -> clamp (va=-1 anyway)

    nc = bacc.Bacc(None, target_bir_lowering=False, debug=False,
                   num_swdge_queues=4)

    # ---- I/O ----
    xa1 = nc.dram_tensor("xa1", [NB, P, TB1 * D], BF16, kind="ExternalInput")
    oha1 = nc.dram_tensor("oha1", [NB, P, TB1 * P], BF16, kind="ExternalInput")
    xa2 = nc.dram_tensor("xa2", [NB, P, TB2 * D], BF16, kind="ExternalInput")
    oha2 = nc.dram_tensor("oha2", [NB, P, TB2 * P], BF16, kind="ExternalInput")
    xc = nc.dram_tensor("xc", [P, ESUP], BF16, kind="ExternalInput")
    ohc = nc.dram_tensor("ohc", [P, ESUP], BF16, kind="ExternalInput")
    gidx = nc.dram_tensor("gidx", [P, NSUP * (SUP // 16)], I16, kind="ExternalInput")
    wselfT = nc.dram_tensor("wselfT", [D, D], BF16, kind="ExternalInput")
    winT = nc.dram_tensor("winT", [D, D], BF16, kind="ExternalInput")
    woutT = nc.dram_tensor("woutT", [D, D], BF16, kind="ExternalInput")
    identb = nc.dram_tensor("identb", [D, D], BF16, kind="ExternalInput")
    iota_in = nc.dram_tensor("iota", [P, P], BF16, kind="ExternalInput")
    biascol = nc.dram_tensor("biascol", [P, 1], F32, kind="ExternalInput")
    y = nc.dram_tensor("y", [P, ESUP], BF16, kind="ExternalOutput")

    with tile.TileContext(nc) as tc:
        with (
            tc.tile_pool(name="const", bufs=1) as cpool,
            tc.tile_pool(name="sbuf", bufs=3) as pool,
            tc.tile_pool(name="cstream", bufs=6) as cpool6,
            tc.tile_pool(name="gopool", bufs=12) as gpool,
            tc.tile_pool(name="small", bufs=4) as spool,
            tc.tile_pool(name="dram", bufs=1, space="DRAM") as dram,
        ):
            nc.gpsimd.load_library(library_config.mlp)
            iota_t = cpool.tile([P, P], BF16)
            nc.sync.dma_start(out=iota_t[:], in_=iota_in[:])
            wselfT_t = cpool.tile([D, D], BF16)
            nc.sync.dma_start(out=wselfT_t[:], in_=wselfT[:])
            winT_t = cpool.tile([D, D], BF16)
            nc.sync.dma_start(out=winT_t[:], in_=winT[:])
            woutT_t = cpool.tile([D, D], BF16)
            nc.sync.dma_start(out=woutT_t[:], in_=woutT[:])
            identb_t = cpool.tile([D, D], BF16)
            nc.sync.dma_start(out=identb_t[:], in_=identb[:])
            biascol_t = cpool.tile([P, 1], F32)
            nc.sync.dma_start(out=biascol_t[:], in_=biascol[:])
            gidx_t = cpool.tile([P, NSUP * (SUP // 16)], I16)
            nc.sync.dma_start(out=gidx_t[:], in_=gidx[:])

            tin_sb = cpool.tile([P, NB * D], BF16)  # T_in slice [n_in_blk, b*D+d]

            cc_in = dram.tile([NBP, D], BF16)
            cc_out = dram.tile([TROWS, D], BF16)

            # ---- phase A (shared): block-uniform one-hot scatter ----
            def phase_a(psum, xa, oha, TB, w_t, out_sb, out_dram, add_bias):
                for b in range(NB):
                    xt = pool.tile([P, TB * D], BF16, tag="xat")
                    nc.sync.dma_start(out=xt[:], in_=xa[b])
                    oht = pool.tile([P, TB * P], BF16, tag="oat")
                    nc.sync.dma_start(out=oht[:], in_=oha[b])
                    sp = psum.tile([P, P], F32, tag="SA")
                    for j in range(TB):
                        nc.tensor.matmul(
                            sp[:], xt[:, j * D : (j + 1) * D],
                            oht[:, j * P : (j + 1) * P],
                            start=(j == 0), stop=(j == TB - 1),
                        )
                    # sp = S^T [d, n]
                    s_sb = spool.tile([P, P], BF16, tag="ssb")
                    nc.scalar.copy(out=s_sb[:], in_=sp[:])
                    tw = psum.tile([P, P], F32, tag="TW")
                    nc.tensor.matmul(tw[:], w_t[:], s_sb[:], start=True, stop=True)
                    # tw = T^T [d', n]
                    t2 = spool.tile([P, P], BF16, tag="t2")
                    if add_bias:
                        nc.vector.tensor_scalar(
                            t2[:], tw[:], biascol_t[:, 0:1], None,
                            mybir.AluOpType.add,
                        )
                    else:
                        nc.scalar.copy(out=t2[:], in_=tw[:])
                    tr = psum.tile([P, P], BF16, tag="TR")
                    nc.tensor.transpose(tr[:], t2[:], identb_t[:])
                    # tr = T [n, d']
                    if out_sb is not None:
                        nc.scalar.copy(out=out_sb[:, b * D : (b + 1) * D], in_=tr[:])
                    else:
                        t3 = spool.tile([P, P], BF16, tag="t3")
                        nc.scalar.copy(out=t3[:], in_=tr[:])
                        nc.sync.dma_start(
                            out=out_dram[b * P : (b + 1) * P, :], in_=t3[:]
                        )

            # A2 first: its AllGather overlaps A1
            with tc.tile_pool(name="psumA", bufs=2, space="PSUM") as psumA:
                phase_a(psumA, xa2, oha2, TB2, woutT_t, None, cc_in, False)
                nc.gpsimd.collective_compute(
                    "AllGather", mybir.AluOpType.bypass,
                    replica_groups=[list(range(C))],
                    ins=[cc_in.opt()], outs=[cc_out.opt()],
                )
                phase_a(psumA, xa1, oha1, TB1, winT_t, tin_sb, None, True)

            # ---- phase C ----
            psum_cm = tc.tile_pool(name="psumC", bufs=2, space="PSUM")
            psum = psum_cm.__enter__()
            tbl_lo = cc_out[0:HALF, :]
            tbl_hi = cc_out[HALF:TROWS, :]
            SIDX = SUP // 16
            for si, s in enumerate(range(NSUP)):
                xcs = cpool6.tile([P, SUP], BF16, tag="xcs")
                nc.sync.dma_start(out=xcs[:], in_=xc[:, s * SUP : (s + 1) * SUP])
                ohst = cpool6.tile([P, SUP], BF16, tag="ohst")
                nc.sync.dma_start(out=ohst[:], in_=ohc[:, s * SUP : (s + 1) * SUP])
                go = gpool.tile([P, SUP], BF16, tag="go")
                nc.gpsimd.dma_gather(
                    out_ap=go[:].rearrange("p (j d) -> p j d", j=SUP // P),
                    in_ap=(tbl_lo if s < NSUP_LO else tbl_hi),
                    idxs_ap=gidx_t[:, s * SIDX : (s + 1) * SIDX],
                    num_idxs=SUP, num_idxs_reg=SUP, elem_size=D,
                    transpose=False, single_packet=False, queue_num=si % 4,
                )
                ysup = pool.tile([P, SUP], BF16, tag="ysup")
                zsb = pool.tile([P, SUP], BF16, tag="zsb")
                for j in range(SUP // P):
                    tg = s * (SUP // P) + j
                    b = tile_block(tg)
                    off = j * P
                    zp = psum.tile([P, P], F32, tag="Z")
                    # z_nat[e, d] = x^T_tile.T @ WselfT + Tin expand
                    nc.tensor.matmul(
                        zp[:], xcs[:, off : off + P], wselfT_t[:],
                        start=True, stop=False,
                    )
                    nc.tensor.matmul(
                        zp[:], ohst[:, off : off + P],
                        tin_sb[:, b * D : (b + 1) * D],
                        start=False, stop=True,
                    )
                    nc.scalar.copy(out=zsb[:, off : off + P], in_=zp[:])
                # + T_out (gathered, same [e_p, j*128+d] layout) on DVE
                zadd = pool.tile([P, SUP], BF16, tag="zadd")
                nc.vector.tensor_add(zadd[:], zsb[:], go[:])
                t1 = spool.tile([P, SUP], BF16, tag="t1")
                nc.scalar.mul(out=t1[:], in_=zadd[:], mul=0.2)
                nc.vector.tensor_max(ysup[:], zadd[:], t1[:])
                nc.sync.dma_start(out=y[:, s * SUP : (s + 1) * SUP], in_=ysup[:])
            psum_cm.__exit__(None, None, None)

    nc.compile()
    return nc


def prepare_inputs(cfg, edge_attr, edge_index, W_self_w, W_self_b, W_in_w, W_out_w):
    import ml_dtypes
    bf16 = ml_dtypes.bfloat16
    E, N = cfg["E"], cfg["N"]
    NPC, NB, NBP = _derive(cfg)

    edge_attr = np.ascontiguousarray(edge_attr, dtype=np.float32)
    src = np.asarray(edge_index[0], dtype=np.int64)
    dst = np.asarray(edge_index[1], dtype=np.int64)

    wselfT = np.ascontiguousarray(np.asarray(W_self_w, np.float32).T).astype(bf16)
    winT = np.ascontiguousarray(np.asarray(W_in_w, np.float32).T).astype(bf16)
    woutT = np.ascontiguousarray(np.asarray(W_out_w, np.float32).T).astype(bf16)
    identb = np.eye(D, dtype=np.float32).astype(bf16)
    iota = np.tile(np.arange(P, dtype=np.float32)[None, :], (P, 1)).astype(bf16)
    biascol = np.asarray(W_self_b, np.float32).reshape(P, 1)

    ea_bf = edge_attr.astype(bf16)

    # per-node inverse counts (torch_scatter mean semantics: clamp >= 1)
    cnt_dst = np.bincount(dst, minlength=N).astype(np.float32)
    cnt_src = np.bincount(src, minlength=N).astype(np.float32)
    inv_dst = 1.0 / np.maximum(cnt_dst, 1.0)
    inv_src = 1.0 / np.maximum(cnt_src, 1.0)

    # ---- block-uniform A-stream builder ----
    def build_a(node_of_edge, inv_node):
        owner = node_of_edge // NPC
        local = node_of_edge - owner * NPC
        blk = local >> 7
        percore = []
        maxtb = 1
        for c in range(C):
            sel = np.nonzero(owner == c)[0]
            order = sel[np.argsort(local[sel], kind="stable")]
            bcnt = np.bincount(blk[order], minlength=NB)
            maxtb = max(maxtb, int(np.ceil(bcnt.max() / P)))
            percore.append((order, bcnt))
        TB = maxtb
        xs = np.zeros((C, NB, P, TB * D), dtype=bf16)
        ohs = np.zeros((C, NB, P, TB * P), dtype=bf16)
        orders = []
        for c in range(C):
            order, bcnt = percore[c]
            starts = np.zeros(NB, dtype=np.int64)
            np.cumsum(bcnt[:-1], out=starts[1:])
            pos = np.arange(len(order)) - starts[blk[order]]
            slot = blk[order] * (TB * P) + pos  # row within padded stream
            x_pad = np.zeros((NB * TB * P, D), dtype=bf16)
            x_pad[slot] = ea_bf[order]
            xs[c] = (x_pad.reshape(NB, TB, P, D).transpose(0, 2, 1, 3)
                     .reshape(NB, P, TB * D))
            oh_pad = np.zeros((NB * TB * P, P), dtype=bf16)
            oh_pad[slot, (local[order] & 127)] = inv_node[node_of_edge[order]]
            ohs[c] = (oh_pad.reshape(NB, TB, P, P).transpose(0, 2, 1, 3)
                      .reshape(NB, P, TB * P))
            orders.append(order)
        return TB, xs, ohs, orders

    TB1, xs1, ohs1, _ = build_a(dst, inv_dst)
    TB2, xs2, ohs2, _ = build_a(src, inv_src)

    trow = lambda n: (n // NPC) * NBP + (n % NPC)
    dst_row = trow(dst)
    hi = (dst_row >= HALF).astype(np.int64)

    # ---- C stream: (half, src-block)-uniform layout ----
    src_owner = src // NPC
    src_local = src - src_owner * NPC
    src_blk = src_local >> 7
    # per (core, block, half) counts -> uniform TLO/THI
    TLO = THI = 1
    perc = []
    for c in range(C):
        sel = np.nonzero(src_owner == c)[0]
        order = sel[np.argsort(src_local[sel] * 2 + hi[sel], kind="stable")]
        blk_o = src_blk[order]
        hi_o = hi[order]
        cl = np.bincount(blk_o[hi_o == 0], minlength=NB)
        ch = np.bincount(blk_o[hi_o == 1], minlength=NB)
        TLO = max(TLO, int(np.ceil(cl.max() / P)))
        THI = max(THI, int(np.ceil(ch.max() / P)))
        perc.append((order, cl, ch))
    NSUP_LO = int(np.ceil(NB * TLO * P / SUP))
    NSUP_HI = int(np.ceil(NB * THI * P / SUP))
    NSUP = NSUP_LO + NSUP_HI
    ESUP = NSUP * SUP
    NTC = ESUP // P
    HIBASE = NSUP_LO * SUP  # stream row where the hi group starts

    xcs = np.zeros((C, P, ESUP), dtype=bf16)
    ohcs = np.zeros((C, P, ESUP), dtype=bf16)
    gidx = np.zeros((C, P, NSUP * (SUP // 16)), dtype=np.int16)
    slots_all = []
    for c in range(C):
        order, cl, ch = perc[c]
        blk_o = src_blk[order]
        hi_o = hi[order]
        slot = np.zeros(len(order), dtype=np.int64)
        for h, cnts, base, TBH in ((0, cl, 0, TLO), (1, ch, HIBASE, THI)):
            idxs = np.nonzero(hi_o == h)[0]
            b_of = blk_o[idxs]
            o2 = np.argsort(b_of, kind="stable")
            bb = b_of[o2]
            st = np.zeros(NB, dtype=np.int64)
            np.cumsum(cnts[:-1], out=st[1:])
            runidx = np.arange(len(idxs)) - st[bb]  # position within block
            ordpos = np.empty(len(idxs), dtype=np.int64)
            ordpos[o2] = runidx
            slot[idxs] = base + b_of * (TBH * P) + ordpos
        x_pad = np.zeros((ESUP, D), dtype=bf16)
        x_pad[slot] = ea_bf[order]
        xcs[c] = x_pad.T
        ohcs[c][(src_local[order] & 127), slot] = 1.0
        gv = np.zeros(ESUP, dtype=np.int64)
        gv[slot] = dst_row[order] - hi[order] * HALF
        gi = np.zeros((16, ESUP // 16), dtype=np.int16)
        gi[np.arange(ESUP) % 16, np.arange(ESUP) // 16] = gv.astype(np.int16)
        gidx[c] = np.tile(gi, (8, 1))
        slots_all.append((order, slot))

    in_maps = []
    for c in range(C):
        in_maps.append(dict(
            xa1=xs1[c], oha1=ohs1[c],
            xa2=xs2[c], oha2=ohs2[c],
            xc=xcs[c], ohc=ohcs[c], gidx=gidx[c],
            wselfT=wselfT, winT=winT, woutT=woutT,
            identb=identb, iota=iota, biascol=biascol,
        ))

    def postprocess(results):
        full = np.empty((E, D), dtype=np.float32)
        for c in range(C):
            yT = np.asarray(results[c]["y"], dtype=np.float32)  # [128, ESUP]
            # natural per-tile layout: yT[p, t*128+d] = z[edge(t,p), d]
            ynat = (yT.reshape(P, yT.shape[1] // P, P).transpose(1, 0, 2)
                    .reshape(-1, P))
            order, slot = slots_all[c]
            full[order] = ynat[slot]
        return full

    return (TB1, TB2, TLO, THI, NSUP_LO, NSUP_HI), in_maps, postprocess


_NC_CACHE = {}


def run(cfg, inputs, trace=False, trace_kwargs=None):
    params, in_maps, post = prepare_inputs(
        cfg,
        inputs["edge_attr"],
        inputs["edge_index"],
        inputs["W_self_w"],
        inputs["W_self_b"],
        inputs["W_in_w"],
        inputs["W_out_w"],
    )
    key = (tuple(sorted(cfg.items())), params)
    if key not in _NC_CACHE:
        _NC_CACHE[key] = build_kernel(cfg, *params)
    nc = _NC_CACHE[key]
    kw = {}
    if trace:
        kw["trace"] = True
        if trace_kwargs:
            kw.update(trace_kwargs)
    res = run_bass_kernel_spmd(nc, in_maps, core_ids=list(range(C)), **kw)
    return post(res.results), res


def kernel(**inputs) -> np.ndarray:
    out, _ = run(_cfg_full(), inputs)
    return out.astype(np.float32)
